# revision 11
# baseline (speedup 1.0000x reference)
"""Self-contained Trainium2 Bass kernel for NemotronH MTP MoE layer.

Expert-parallel over 8 NeuronCores: core c owns experts [8c, 8c+8); the
shared-expert MLP is tensor-parallel sliced (256 of 2048 intermediate dims
per core).  The DeepSeekV3-style gate is computed host-side (tiny), tokens
are dispatched host-side into per-expert column blocks with the combine
weight folded in as sqrt(w) (exact: relu^2 is degree-2 homogeneous), and
each core scatter-adds its experts' outputs into its [T, H] partial with
indirect accumulate-DMA.  The host sums the 8 partials (the expert-parallel
unshard/combine).

Matmuls run in float32r (TF32 path, full PE rate at N=512, ~1.5e-4 rel
err), accumulating in fp32 PSUM.  f32r cannot be an I/O dtype, so inputs
arrive fp32 and are cast on-device with DVE copies.
"""

import sys

sys.path.insert(0, "/opt/trn_rl_repo")

import numpy as np

# ---- problem constants (hardcoded per contract) ----
B, S, H = 2, 512, 2048
E, G, TOPK_G, K = 64, 8, 4, 6
I = 512
SH_I = 2048
RSF = 2.5
T = B * S  # 1024 tokens
N_CORES = 8
EL = E // N_CORES  # 8 experts per core
SH_SL = SH_I // N_CORES  # 256 shared-intermediate dims per core
P = 128
KH = H // P  # 16 K-tiles over hidden
KI = I // P  # 4 K-tiles over expert intermediate
OOB = 1 << 27  # padded scatter index -> skipped via bounds_check

_PROG_CACHE = {}


def _gate_numpy(x, gate_w, gate_bias):
    """noaux_tc gate: sigmoid+bias, group top-2 sum, top-4 groups, top-6."""
    logits = x @ gate_w.T
    scores = 1.0 / (1.0 + np.exp(-logits))
    scores_b = scores + gate_bias
    sb_g = scores_b.reshape(T, G, E // G)
    top2 = np.sort(sb_g, axis=-1)[..., -2:].sum(-1, dtype=np.float32)
    grp_idx = np.argsort(-top2, axis=-1, kind="stable")[:, :TOPK_G]
    grp_mask = np.zeros((T, G), np.float32)
    np.put_along_axis(grp_mask, grp_idx, 1.0, axis=1)
    expert_mask = np.repeat(grp_mask, E // G, axis=-1) > 0
    masked = np.where(expert_mask, scores_b, -np.inf)
    top_idx = np.argsort(-masked, axis=1, kind="stable")[:, :K]
    topw = np.take_along_axis(scores, top_idx, axis=1)
    topw = topw / (topw.sum(-1, keepdims=True, dtype=np.float32) + 1e-20) * RSF
    return top_idx, topw.astype(np.float32)


def _build_program(nslot):
    """Build + compile the SPMD Bass program. nslot = 128-row M-tiles per
    expert (1 unless some expert holds >128 tokens)."""
    import concourse.bass as bass
    import concourse.tile as tile
    from concourse import bacc, mybir
    from concourse.masks import make_identity

    f32 = mybir.dt.float32
    f32r = mybir.dt.float32r
    i32 = mybir.dt.int32
    Relu = mybir.ActivationFunctionType.Relu

    NV = EL * nslot  # virtual experts (one 128-token M-tile each)

    nc = bacc.Bacc("TRN2", target_bir_lowering=False, debug=False, num_devices=N_CORES)

    xt = nc.dram_tensor("xt", [H, T], f32, kind="ExternalInput").ap()
    xst = nc.dram_tensor("xst", [H, NV * P], f32, kind="ExternalInput").ap()
    w1t = nc.dram_tensor("w1t", [EL, H, I], f32, kind="ExternalInput").ap()
    w2t = nc.dram_tensor("w2t", [EL, I, H], f32, kind="ExternalInput").ap()
    shupt = nc.dram_tensor("shupt", [H, SH_SL], f32, kind="ExternalInput").ap()
    shdownt = nc.dram_tensor("shdownt", [SH_SL, H], f32, kind="ExternalInput").ap()
    idx = nc.dram_tensor("idx", [P, NV], i32, kind="ExternalInput").ap()
    out = nc.dram_tensor("out", [T, H], f32, kind="ExternalOutput").ap()

    with tile.TileContext(nc) as tc:
        with (
            tc.tile_pool(name="p_xs", bufs=2) as p_xs,  # per-expert tokens
            tc.tile_pool(name="p_stage", bufs=5) as p_stage,  # fp32 staging [P, H]
            tc.tile_pool(name="p_xt", bufs=3) as p_xt,
            tc.tile_pool(name="p_shupt", bufs=2) as p_shupt,
            tc.tile_pool(name="p_shdownt", bufs=1) as p_shdownt,  # tags sd0/sd1
            tc.tile_pool(name="p_actsh", bufs=1) as p_actsh,  # tags actsh0/1
            tc.tile_pool(name="p_w1", bufs=6) as p_w1,
            tc.tile_pool(name="p_w2", bufs=3) as p_w2,
            tc.tile_pool(name="p_tmp", bufs=2) as p_tmp,
            tc.tile_pool(name="p_actT", bufs=8) as p_actT,
            tc.tile_pool(name="p_y", bufs=2) as p_y,  # tags o_sh / y_e
            tc.tile_pool(name="p_small", bufs=1) as p_small,
            tc.tile_pool(name="ps_all", bufs=4, space="PSUM") as ps_all,  # tag psA
            tc.tile_pool(name="ps_up", bufs=2, space="PSUM") as ps_up,  # tag pu
            tc.tile_pool(name="ps_tr", bufs=2, space="PSUM") as ps_tr,  # tag pt
        ):

            def load_cast(pool, dram_slice, shape, name):
                """DMA fp32 DRAM -> staging, DVE cast -> f32r tile."""
                st = p_stage.tile([P, H], f32, name="stage")
                nc.sync.dma_start(st[: shape[0], : shape[1]], dram_slice)
                tl = pool.tile(list(shape), f32r, name=name)
                nc.vector.tensor_copy(tl[:], st[: shape[0], : shape[1]])
                return tl

            # ---- constants / small loads ----
            ident = p_small.tile([P, P], f32, name="ident")
            make_identity(nc, ident[:])
            tidx = p_small.tile([P, NV], i32, name="tidx")
            nc.sync.dma_start(tidx[:], idx[:])

            # ================= shared MLP (TP slice) =================
            act_shT = [p_actsh.tile([P, T], f32r, name=f"actsh{m}") for m in range(2)]
            ps_sh = [[None, None], [None, None]]
            for m in range(2):
                for nch in range(2):
                    ps_sh[m][nch] = ps_all.tile([P, 512], f32, name="psA")
            for k in range(KH):
                xt_k = load_cast(p_xt, xt[k * P : (k + 1) * P, :], (P, T), "xt_k")
                su_k = load_cast(
                    p_shupt, shupt[k * P : (k + 1) * P, :], (P, SH_SL), "su_k"
                )
                for m in range(2):
                    for nch in range(2):
                        nc.tensor.matmul(
                            ps_sh[m][nch][:],
                            su_k[:, m * P : (m + 1) * P],
                            xt_k[:, nch * 512 : (nch + 1) * 512],
                            start=(k == 0),
                            stop=(k == KH - 1),
                        )
            for m in range(2):
                for nch in range(2):
                    pp = ps_sh[m][nch]
                    r = p_tmp.tile([P, 512], f32, name="r_sh")
                    nc.scalar.activation(r[:], pp[:], Relu, 0.0, 1.0, 0.0)
                    t2 = p_tmp.tile([P, 512], f32, name="t2_sh")
                    nc.vector.tensor_tensor(
                        out=t2[:], in0=pp[:], in1=r[:], op=mybir.AluOpType.mult
                    )
                    nc.vector.tensor_copy(
                        act_shT[m][:, nch * 512 : (nch + 1) * 512], t2[:]
                    )

            # down: out[t, :] = act_shT.T @ shdownt  (accumulate over 2 k2)
            sd = [
                load_cast(
                    p_shdownt, shdownt[k2 * P : (k2 + 1) * P, :], (P, H), f"sd{k2}"
                )
                for k2 in range(2)
            ]
            for mt in range(T // P):
                o_sh = p_y.tile([P, H], f32, name="o_sh")
                pss = [ps_all.tile([P, 512], f32, name="psA") for h in range(4)]
                for k2 in range(2):
                    for hch in range(4):
                        nc.tensor.matmul(
                            pss[hch][:],
                            act_shT[k2][:, mt * P : (mt + 1) * P],
                            sd[k2][:, hch * 512 : (hch + 1) * 512],
                            start=(k2 == 0),
                            stop=(k2 == 1),
                        )
                for hch in range(4):
                    nc.vector.tensor_copy(
                        o_sh[:, hch * 512 : (hch + 1) * 512], pss[hch][:]
                    )
                nc.sync.dma_start(out[mt * P : (mt + 1) * P, :], o_sh[:])

            # ================= routed experts =================
            for v in range(NV):
                e = v // nslot
                # per-expert gathered tokens, all 16 K-tiles in one DMA:
                # xst[:, v*128:(v+1)*128] = [2048, 128] -> [128, 16*128]
                st = p_stage.tile([P, H], f32, name="stage")
                nc.sync.dma_start(
                    st[:, : KH * P].rearrange("p (k c) -> p k c", c=P),
                    xst[:, v * P : (v + 1) * P].rearrange("(k p) c -> p k c", p=P),
                )
                xs_e = p_xs.tile([P, KH * P], f32r, name="xs_e")
                nc.vector.tensor_copy(xs_e[:], st[:, : KH * P])
                # --- up-projection: psum [128 tok, 512 I] ---
                pu = ps_up.tile([P, I], f32, name="pu")
                for k in range(KH):
                    w1_k = load_cast(
                        p_w1, w1t[e, k * P : (k + 1) * P, :], (P, I), "w1_k"
                    )
                    nc.tensor.matmul(
                        pu[:],
                        xs_e[:, k * P : (k + 1) * P],
                        w1_k[:],
                        start=(k == 0),
                        stop=(k == KH - 1),
                    )
                # --- relu2 ---
                r = p_tmp.tile([P, I], f32, name="r_e")
                nc.scalar.activation(r[:], pu[:], Relu, 0.0, 1.0, 0.0)
                act = p_tmp.tile([P, I], f32, name="act_e")
                nc.vector.tensor_tensor(
                    out=act[:], in0=pu[:], in1=r[:], op=mybir.AluOpType.mult
                )
                # --- transpose act -> actT (4 x [128 I, 128 tok], f32r) ---
                actT = []
                for it in range(KI):
                    pt = ps_tr.tile([P, P], f32, name="pt")
                    nc.tensor.transpose(pt[:], act[:, it * P : (it + 1) * P], ident[:])
                    at = p_actT.tile([P, P], f32r, name="at")
                    nc.vector.tensor_copy(at[:], pt[:])
                    actT.append(at)
                # --- down-projection: 4 psums [128 tok, 512 H-chunk] ---
                pd = [ps_all.tile([P, 512], f32, name="psA") for h in range(4)]
                for it in range(KI):
                    w2_i = load_cast(
                        p_w2, w2t[e, it * P : (it + 1) * P, :], (P, H), "w2_i"
                    )
                    for hch in range(4):
                        nc.tensor.matmul(
                            pd[hch][:],
                            actT[it][:],
                            w2_i[:, hch * 512 : (hch + 1) * 512],
                            start=(it == 0),
                            stop=(it == KI - 1),
                        )
                y = p_y.tile([P, H], f32, name="y_e")
                for hch in range(4):
                    nc.vector.tensor_copy(y[:, hch * 512 : (hch + 1) * 512], pd[hch][:])
                # --- scatter-add into out rows (pads are OOB -> skipped) ---
                nc.gpsimd.indirect_dma_start(
                    out=out[:],
                    out_offset=bass.IndirectOffsetOnAxis(ap=tidx[:, v : v + 1], axis=0),
                    in_=y[:],
                    in_offset=None,
                    compute_op=mybir.AluOpType.add,
                    bounds_check=T - 1,
                    oob_is_err=False,
                )

    nc.compile()
    return nc


def _prepare(inputs):
    """Host gate + dispatch: returns (nc, in_maps) ready for SPMD dispatch."""
    hidden_states = np.asarray(inputs["hidden_states"], dtype=np.float32)
    gate_w = np.asarray(inputs["gate_w"], dtype=np.float32)
    gate_bias = np.asarray(inputs["gate_bias"], dtype=np.float32)
    w1 = np.asarray(inputs["w1"], dtype=np.float32)
    w2 = np.asarray(inputs["w2"], dtype=np.float32)
    shared_up = np.asarray(inputs["shared_up"], dtype=np.float32)
    shared_down = np.asarray(inputs["shared_down"], dtype=np.float32)

    x = hidden_states.reshape(T, H)

    # ---- host gate + dispatch ----
    top_idx, topw = _gate_numpy(x, gate_w, gate_bias)
    sqw = np.sqrt(topw)

    tok_lists = [[] for _ in range(E)]
    scale_lists = [[] for _ in range(E)]
    for kk in range(K):
        for t in range(T):
            e = top_idx[t, kk]
            tok_lists[e].append(t)
            scale_lists[e].append(sqw[t, kk])
    counts = np.array([len(l) for l in tok_lists])
    nslot = max(1, int(np.ceil(counts.max() / P)))

    if nslot not in _PROG_CACHE:
        _PROG_CACHE[nslot] = _build_program(nslot)
    nc = _PROG_CACHE[nslot]

    NV = EL * nslot
    CAP = nslot * P

    xt_np = np.ascontiguousarray(x.T)  # [H, T]

    in_maps = []
    for c in range(N_CORES):
        xst_c = np.zeros((H, NV * P), np.float32)
        idx_c = np.full((P, NV), OOB, np.int32)
        for j in range(EL):
            e = c * EL + j
            toks = np.array(tok_lists[e], dtype=np.int64)
            scls = np.array(scale_lists[e], dtype=np.float32)
            n = len(toks)
            assert n <= CAP
            if n:
                xs = x[toks] * scls[:, None]  # [n, H]
                xst_c[:, j * CAP : j * CAP + n] = xs.T
                for s in range(nslot):
                    lo, hi = s * P, min((s + 1) * P, n)
                    if lo >= n:
                        break
                    idx_c[: hi - lo, j * nslot + s] = toks[lo:hi]
        in_maps.append(
            {
                "xt": xt_np,
                "xst": np.ascontiguousarray(xst_c),
                "w1t": np.ascontiguousarray(
                    w1[c * EL : (c + 1) * EL].transpose(0, 2, 1)
                ),
                "w2t": np.ascontiguousarray(
                    w2[c * EL : (c + 1) * EL].transpose(0, 2, 1)
                ),
                "shupt": np.ascontiguousarray(
                    shared_up.T[:, c * SH_SL : (c + 1) * SH_SL]
                ),
                "shdownt": np.ascontiguousarray(
                    shared_down.T[c * SH_SL : (c + 1) * SH_SL, :]
                ),
                "idx": idx_c,
            }
        )

    return nc, in_maps


def kernel(**inputs):
    from concourse.bass_utils import run_bass_kernel_spmd

    hidden_states = np.asarray(inputs["hidden_states"], dtype=np.float32)
    nc, in_maps = _prepare(inputs)
    res = run_bass_kernel_spmd(nc, in_maps, list(range(N_CORES)))

    acc = np.zeros((T, H), np.float32)
    for c in range(N_CORES):
        acc += res.results[c]["out"]
    return acc.reshape(hidden_states.shape).astype(hidden_states.dtype)
